# revision 6
# baseline (speedup 1.0000x reference)
"""Causal single-head attention block on 8 TRN2 NeuronCores.

Reference: Q=x@Wq, K=x@Wk, V=x@Wv; S=Q@K^T (no pre-softmax scaling);
causal mask; P=softmax(S); out=(P@V)/sqrt(64).
Shapes: x [4, 2048, 1024] f32, W* [1024, 64] f32 -> out [4, 2048, 64].

Sharding: 8 cores = 4 batches x 2 interleaved query-tile sets.
Core (b, jj) owns 8 query tiles of 128 rows; both sets have equal
causal work at 128-key granularity. Host permutes x[b]^T (fp16) into
"slots" INTERLEAVED own/comp: slot 2p = own tile p, 2p+1 = comp tile p
for p<6; then own 6 (slot 12), own 7 (13), comp 6 (14), comp 7 (15).
Each arriving pair completes one full E row, so exp (ACT) load spreads
evenly instead of bunching after the last slots, while the final slot
(comp 7) still feeds only one last S^T block.

Causality = shared 128x128 triangular mask on the diagonal block +
per-core 0/1 scalar folded into a pre-scaled boundary V slot ("vz").

Per pair p (own slot 2p, comp slot 2p+1), one [128,3,128] psum tile:
  col0 = [Wk|Wq]^T @ x_own^T  (fused)     -> kt / Q^T
  col1 = [Wk|Wv/8]^T @ x_comp^T (fused)   -> kt / V^T (rides free)
  col2 = x_own^T.T @ (Wv/8)   (natural V for own)
  kt pair copy (rows 0:64) + q|v pair copy (rows 64:128) on DVE;
  V^T -> V via 64-col PE transpose (identity operand);
  S^T [t,q] blocks -> exp chunks of up to 8 blocks (ACT);
  E diag *= tri (Pool); vz (Pool); PV accumulates [V | 1] into a
  [128,2,65] psum pair; raw 65-col result DMA'd out; host divides.
PV(p-1) is emitted right after pair p's copies so the PE chews on it
while DVE copies land (lag-1 pipeline).
"""

import sys

import numpy as np
import ml_dtypes

try:  # concourse ships in the TRN container; fall back to its known path
    import concourse  # noqa: F401
except ImportError:
    sys.path.insert(0, "/opt/trn_rl_repo")

B, T, C, DK = 4, 2048, 1024, 64
NLI = 8          # query tiles per core
NSLOT = 16       # key tiles (slots) per batch

OWN = [0, 2, 4, 6, 8, 10, 12, 13]   # slot of own query tile li
CMP = [1, 3, 5, 7, 9, 11, 14, 15]   # slot of comp tile k

_CACHE = {}


def _build():
    import concourse.bacc as bacc
    import concourse.tile as tile
    import concourse.mybir as mybir

    f32 = mybir.dt.float32
    f16 = mybir.dt.float16
    bf16 = mybir.dt.bfloat16
    EXP = mybir.ActivationFunctionType.Exp
    CPY = mybir.ActivationFunctionType.Copy

    nc = bacc.Bacc("TRN2", target_bir_lowering=False, debug=False,
                   enable_asserts=False, num_devices=8)

    xt_d = nc.dram_tensor("xt", [128, NSLOT, 1024], f16,
                          kind="ExternalInput").ap()
    wa_d = nc.dram_tensor("wa", [128, 8, 2, 64], f16,
                          kind="ExternalInput").ap()
    wb_d = nc.dram_tensor("wb", [128, 8, 2, 64], f16,
                          kind="ExternalInput").ap()
    idt_d = nc.dram_tensor("idt", [64, 64], f16, kind="ExternalInput").ap()
    msk_d = nc.dram_tensor("msk", [128, 136], bf16,
                           kind="ExternalInput").ap()
    y_d = nc.dram_tensor("y", [128, NLI, DK + 1], f32,
                         kind="ExternalOutput").ap()

    with tile.TileContext(nc) as tc:
        with (
            tc.tile_pool(name="persist", bufs=1) as pp,
            tc.tile_pool(name="pmix", bufs=2, space="PSUM") as pmx,
            tc.tile_pool(name="ptrp", bufs=1, space="PSUM") as ptrp,
            tc.tile_pool(name="pst", bufs=2, space="PSUM") as pst,
            tc.tile_pool(name="pout", bufs=1, space="PSUM") as pou,
        ):
            xt = pp.tile([128, NSLOT, 1024], f16, tag="xt", name="xt")
            wa = pp.tile([128, 8, 2, 64], f16, tag="wa", name="wa")
            wb = pp.tile([128, 8, 2, 64], f16, tag="wb", name="wb")
            idt = pp.tile([64, 64], f16, tag="idt", name="idt")
            msk = pp.tile([128, 136], bf16, tag="msk", name="msk")
            tri = msk[:, 0:128]
            svec = pp.tile([128, NLI], f32, tag="svec", name="svec")
            kt = pp.tile([64, NSLOT, 128], f16, tag="kt", name="kt")
            # qvt[:, i, 0, :] = Q^T of own li=i; [:, i, 1, :] = V^T of
            # comp k=i (transpose staging)
            qvt = pp.tile([64, NLI, 2, 128], f16, tag="qvt", name="qvt")
            vv = pp.tile([128, NSLOT, DK + 1], bf16, tag="vv", name="vv")
            vz = pp.tile([128, NLI, DK + 1], bf16, tag="vz", name="vz")
            po_sb = pp.tile([128, NLI, DK + 1], f32, tag="posb", name="posb")
            scr = pp.tile([128, 128], f16, tag="scr", name="scr")
            E = [pp.tile([128, (2 * li + 2) * 128], bf16, tag=f"E{li}",
                         name=f"E{li}") for li in range(NLI)]

            nc.vector.memset(scr, 0.0)
            nc.vector.memset(vv[:, :, DK:DK + 1], 1.0)

            # ---- DMA program (all SP; transfers serialize in order) ----
            nc.sync.dma_start(wa, wa_d)
            nc.sync.dma_start(xt[:, 0, 0:512], xt_d[:, 0, 0:512])
            nc.sync.dma_start(xt[:, 0, 512:1024], xt_d[:, 0, 512:1024])
            nc.sync.dma_start(wb, wb_d)
            nc.sync.dma_start(xt[:, 1, :], xt_d[:, 1, :])
            nc.sync.dma_start(msk, msk_d)
            nc.sync.dma_start(idt, idt_d)
            for a, b in [(2, 4), (4, 6), (6, 8), (8, 10), (10, 12),
                         (12, 14)]:
                nc.sync.dma_start(xt[:, a:b, :], xt_d[:, a:b, :])
            nc.sync.dma_start(xt[:, 14, :], xt_d[:, 14, :])
            nc.sync.dma_start(xt[:, 15, 0:512], xt_d[:, 15, 0:512])
            nc.sync.dma_start(xt[:, 15, 512:1024], xt_d[:, 15, 512:1024])

            # ---- minimal PE warmup (sets pe_busy_start ~1.2us) ----
            for _ in range(2):
                pw = pmx.tile([128, 3, 128], f32, tag="pmix", name="pw")
                nc.tensor.matmul(pw[:, 0, 0:2], scr, scr[:, 0:2],
                                 start=True, stop=True)

            ps_of = {}

            def ps_pair(p):
                if p not in ps_of:
                    ps_of[p] = pmx.tile([128, 3, 128], f32, tag="pmix",
                                        name=f"pp{p}")
                return ps_of[p]

            def fused(p, col, wt, s, chunks=tuple(range(8)), first=True,
                      last=True):
                ps = ps_pair(p)
                for i, ch in enumerate(chunks):
                    nc.tensor.matmul(
                        ps[:, col, :], wt[:, ch, :, :],
                        xt[:, s, ch * 128:(ch + 1) * 128],
                        start=(first and i == 0),
                        stop=(last and i == len(chunks) - 1))

            def vnat(p, s, chunks=tuple(range(8)), first=True, last=True):
                ps = ps_pair(p)
                for i, ch in enumerate(chunks):
                    nc.tensor.matmul(
                        ps[:, 2, 0:DK],
                        xt[:, s, ch * 128:(ch + 1) * 128],
                        wb[:, ch, 1, :],
                        start=(first and i == 0),
                        stop=(last and i == len(chunks) - 1))

            def cp_pair(p, vv_eng=None):
                """kt pair + q|v pair + vv-own copies for full pair p."""
                ps = ps_of[p]
                s0 = 2 * p
                nc.vector.tensor_copy(kt[:, s0:s0 + 2, :],
                                      ps[0:64, 0:2, :])
                nc.vector.tensor_copy(qvt[:, p, :, :], ps[64:128, 0:2, :])
                if vv_eng == "act":
                    nc.scalar.activation(vv[:, s0, 0:DK], ps[:, 2, 0:DK],
                                         CPY)
                else:
                    nc.vector.tensor_copy(vv[:, s0, 0:DK], ps[:, 2, 0:DK])

            def tr_comp(k):
                """PE transpose V^T(comp k) -> natural V in vv[CMP[k]]."""
                pt = ptrp.tile([128, 2, DK], f16, tag="ptrp", name=f"pt{k}")
                nc.tensor.transpose(pt[:, k % 2, :], qvt[:, k, 1, :], idt)
                nc.vector.tensor_copy(vv[:, CMP[k], 0:DK], pt[:, k % 2, :])

            def s_exp(li, blocks, tag=""):
                """S^T then exp for E[li] col blocks `blocks`."""
                nb = len(blocks)
                ps = pst.tile([128, 1024], f32, tag="pst",
                              name=f"ps{li}{tag}")
                for i, j in enumerate(blocks):
                    s = OWN[j] if j <= li else CMP[j - li - 1]
                    nc.tensor.matmul(
                        ps[:, i * 128:(i + 1) * 128],
                        kt[:, s, :],
                        qvt[:, li, 0, :],
                        start=True, stop=True,
                    )
                j0 = blocks[0]
                nc.scalar.activation(
                    E[li][:, j0 * 128:(j0 + nb) * 128], ps[:, 0:nb * 128],
                    EXP)

            def tri_mul(li):
                nc.gpsimd.tensor_mul(
                    E[li][:, li * 128:(li + 1) * 128],
                    E[li][:, li * 128:(li + 1) * 128], tri)

            def vz_make(li):
                nc.gpsimd.tensor_scalar_mul(
                    vz[:, li, :], vv[:, CMP[li], :], svec[:, li:li + 1])

            po_t = {}

            def pv_mm(li, blocks, start, stop):
                pr = li // 2
                if pr not in po_t:
                    po_t[pr] = pou.tile([128, 2, DK + 1], f32, tag="pout",
                                        name=f"po{pr}")
                po = po_t[pr][:, li % 2, :]
                last = blocks[-1]
                for j in blocks:
                    if j == 2 * li + 1:
                        rhs = vz[:, li, :]
                    else:
                        s = OWN[j] if j <= li else CMP[j - li - 1]
                        rhs = vv[:, s, :]
                    nc.tensor.matmul(
                        po, E[li][:, j * 128:(j + 1) * 128], rhs,
                        start=(start and j == blocks[0]),
                        stop=(stop and j == last),
                        skip_group_check=True,
                    )

            def pv(li):
                pv_mm(li, list(range(2 * li + 2)), True, True)

            def po_cp(li0, n):
                # copy n li-columns of PV psum to SBUF staging (DVE)
                pr = li0 // 2
                c0 = li0 % 2
                nc.vector.tensor_copy(po_sb[:, li0:li0 + n, :],
                                      po_t[pr][:, c0:c0 + n, :])

            # ================= main schedule =================
            # pair 0 (slots 0, 1): own x arrives in halves
            fused(0, 0, wa, 0, (0, 1, 2, 3), True, False)
            fused(0, 0, wa, 0, (4, 5, 6, 7), False, True)
            vnat(0, 0)
            fused(0, 1, wb, 1)
            cp_pair(0, "act")
            nc.vector.tensor_copy(svec, msk[:, 128:136])  # bf16 -> f32
            s_exp(0, [0, 1])
            tr_comp(0)
            tri_mul(0)
            vz_make(0)
            # pair 1 (slots 2, 3)
            fused(1, 0, wa, 2)
            fused(1, 1, wb, 3)
            vnat(1, 2)
            cp_pair(1, "act")
            pv(0)
            s_exp(1, [0, 1, 2, 3])
            tr_comp(1)
            tri_mul(1)
            vz_make(1)
            # pair 2 (slots 4, 5)
            fused(2, 0, wa, 4)
            fused(2, 1, wb, 5)
            vnat(2, 4)
            cp_pair(2, "act")
            pv(1)
            s_exp(2, [0, 1, 2, 3, 4, 5])
            tr_comp(2)
            tri_mul(2)
            vz_make(2)
            po_cp(0, 2)
            nc.sync.dma_start(y_d[:, 0:2, :], po_sb[:, 0:2, :])
            # pair 3 (slots 6, 7)
            fused(3, 0, wa, 6)
            fused(3, 1, wb, 7)
            vnat(3, 6)
            cp_pair(3, "act")
            pv(2)
            s_exp(3, list(range(8)))
            tr_comp(3)
            tri_mul(3)
            vz_make(3)
            # pair 4 (slots 8, 9)
            fused(4, 0, wa, 8)
            fused(4, 1, wb, 9)
            vnat(4, 8)
            cp_pair(4, "act")
            pv(3)
            s_exp(4, list(range(8)), "a")
            s_exp(4, [8, 9], "b")
            tr_comp(4)
            tri_mul(4)
            vz_make(4)
            po_cp(2, 2)
            nc.sync.dma_start(y_d[:, 2:4, :], po_sb[:, 2:4, :])
            # pair 5 (slots 10, 11)
            fused(5, 0, wa, 10)
            fused(5, 1, wb, 11)
            vnat(5, 10)
            cp_pair(5)
            pv(4)
            s_exp(5, list(range(8)), "a")
            s_exp(5, [8, 9, 10, 11], "b")
            tr_comp(5)
            tri_mul(5)
            vz_make(5)
            # own 6 (slot 12)
            fused(6, 0, wa, 12)
            vnat(6, 12)
            ps6 = ps_of[6]
            nc.vector.tensor_copy(kt[:, 12, :], ps6[0:64, 0, :])
            nc.vector.tensor_copy(qvt[:, 6, 0, :], ps6[64:128, 0, :])
            nc.vector.tensor_copy(vv[:, 12, 0:DK], ps6[:, 2, 0:DK])
            pv(5)
            s_exp(6, list(range(8)), "a")
            s_exp(6, [8, 9, 10, 11, 12], "b")
            tri_mul(6)
            po_cp(4, 2)
            nc.sync.dma_start(y_d[:, 4:6, :], po_sb[:, 4:6, :])
            # own 7 (slot 13)
            fused(7, 0, wa, 13)
            vnat(7, 13)
            ps7 = ps_of[7]
            nc.vector.tensor_copy(kt[:, 13, :], ps7[0:64, 0, :])
            nc.vector.tensor_copy(qvt[:, 7, 0, :], ps7[64:128, 0, :])
            nc.vector.tensor_copy(vv[:, 13, 0:DK], ps7[:, 2, 0:DK])
            s_exp(7, list(range(8)), "a")
            s_exp(7, [8, 9, 10, 11, 12, 13], "b")
            tri_mul(7)
            # comp 6 (slot 14)
            fused(8, 1, wb, 14)
            ps8 = ps_of[8]
            nc.vector.tensor_copy(kt[:, 14, :], ps8[0:64, 1, :])
            nc.vector.tensor_copy(qvt[:, 6, 1, :], ps8[64:128, 1, :])
            s_exp(6, [13], "c")
            tr_comp(6)
            vz_make(6)
            s_exp(7, [14], "c")
            pv(6)
            po_cp(6, 1)
            nc.sync.dma_start(y_d[:, 6:7, :], po_sb[:, 6:7, :])
            pv_mm(7, list(range(15)), True, False)
            # comp 7 (slot 15, split by x halves)
            fused(9, 1, wb, 15, (0, 1, 2, 3), True, False)
            fused(9, 1, wb, 15, (4, 5, 6, 7), False, True)
            ps9 = ps_of[9]
            nc.vector.tensor_copy(kt[:, 15, :], ps9[0:64, 1, :])
            nc.vector.tensor_copy(qvt[:, 7, 1, :], ps9[64:128, 1, :])
            s_exp(7, [15], "d")
            tr_comp(7)
            vz_make(7)
            pv_mm(7, [15], False, True)
            po_cp(7, 1)
            nc.sync.dma_start(y_d[:, 7:8, :], po_sb[:, 7:8, :])

    nc.compile()
    return nc


def _host_inputs(x, Wq, Wk, Wv):
    """Per-core input maps. Core c = 2*b + jj."""
    x16 = x.astype(np.float16)
    wk16 = Wk.astype(np.float16).reshape(8, 128, DK).transpose(1, 0, 2)
    wq16 = Wq.astype(np.float16).reshape(8, 128, DK).transpose(1, 0, 2)
    wv16 = (Wv / 8.0).astype(np.float16).reshape(8, 128, DK).transpose(
        1, 0, 2)
    wa_h = np.empty((128, 8, 2, DK), dtype=np.float16)
    wa_h[:, :, 0, :] = wk16
    wa_h[:, :, 1, :] = wq16
    wb_h = np.empty((128, 8, 2, DK), dtype=np.float16)
    wb_h[:, :, 0, :] = wk16
    wb_h[:, :, 1, :] = wv16
    idt = np.eye(64, dtype=np.float16)
    tri = (np.arange(128)[:, None] <= np.arange(128)[None, :])
    in_maps = []
    for core in range(8):
        b, jj = divmod(core, 2)
        sel = [int(k >= 4) if jj == 0 else int(k < 4) for k in range(8)]
        g = [2 * k + sel[k] for k in range(8)]
        cg = [2 * k + 1 - sel[k] for k in range(8)]
        slot_order = [0] * NSLOT
        for li in range(NLI):
            slot_order[OWN[li]] = g[li]
        for k in range(NLI):
            slot_order[CMP[k]] = cg[k]
        arr = x16[b].reshape(16, 128, 8, 128)         # [tile, r, ch, p]
        xt = np.ascontiguousarray(
            arr[slot_order].transpose(3, 0, 2, 1).reshape(128, NSLOT, 1024))
        msk = np.zeros((128, 136), dtype=np.float32)
        msk[:, 0:128] = tri
        msk[:, 128:136] = np.asarray(sel, dtype=np.float32)
        in_maps.append({
            "xt": xt,
            "wa": wa_h,
            "wb": wb_h,
            "idt": idt,
            "msk": msk.astype(ml_dtypes.bfloat16),
        })
    return in_maps


def kernel(x, Wq, Wk, Wv):
    from concourse.bass_utils import run_bass_kernel_spmd

    x = np.asarray(x, dtype=np.float32)
    Wq = np.asarray(Wq, dtype=np.float32)
    Wk = np.asarray(Wk, dtype=np.float32)
    Wv = np.asarray(Wv, dtype=np.float32)

    if "nc" not in _CACHE:
        _CACHE["nc"] = _build()
    nc = _CACHE["nc"]

    in_maps = _host_inputs(x, Wq, Wk, Wv)
    res = run_bass_kernel_spmd(nc, in_maps, core_ids=list(range(8)))
    out = np.empty((B, T, DK), dtype=np.float32)
    for core in range(8):
        b, jj = divmod(core, 2)
        sel = [int(k >= 4) if jj == 0 else int(k < 4) for k in range(8)]
        yloc = res.results[core]["y"]                 # [128, 8, 65]
        for li in range(NLI):
            gt = 2 * li + sel[li]
            out[b, gt * 128:(gt + 1) * 128, :] = (
                yloc[:, li, 0:DK] / yloc[:, li, DK:DK + 1])
    return out


# revision 7
# speedup vs baseline: 1.0739x; 1.0739x over previous
"""Causal single-head attention block on 8 TRN2 NeuronCores.

Reference: Q=x@Wq, K=x@Wk, V=x@Wv; S=Q@K^T (no pre-softmax scaling);
causal mask; P=softmax(S); out=(P@V)/sqrt(64).
Shapes: x [4, 2048, 1024] f32, W* [1024, 64] f32 -> out [4, 2048, 64].

Sharding: 8 cores = 4 batches x 2 interleaved query-tile sets.
Core (b, jj) owns 8 query tiles of 128 rows; both sets have equal
causal work at 128-key granularity. Host permutes x[b]^T (fp16) into
"slots" INTERLEAVED own/comp: slot 2p = own tile p, 2p+1 = comp tile p
for p<6; then own 6 (slot 12), own 7 (13), comp 6 (14), comp 7 (15).
Each arriving pair completes one full E row, so exp (ACT) load spreads
evenly instead of bunching after the last slots, while the final slot
(comp 7) still feeds only one last S^T block.

Causality = shared 128x128 triangular mask on the diagonal block +
per-core 0/1 scalar folded into a pre-scaled boundary V slot ("vz").

Per pair p (own slot 2p, comp slot 2p+1), one [128,3,128] psum tile:
  col0 = [Wk|Wq]^T @ x_own^T  (fused)     -> kt / Q^T
  col1 = [Wk|Wv/8]^T @ x_comp^T (fused)   -> kt / V^T (rides free)
  col2 = x_own^T.T @ (Wv/8)   (natural V for own)
  kt pair copy (rows 0:64) + q|v pair copy (rows 64:128) on DVE;
  V^T -> V via 64-col PE transpose (identity operand);
  S^T [t,q] blocks -> exp chunks of up to 8 blocks (ACT);
  E diag *= tri (Pool); vz (Pool); PV accumulates [V | 1] into a
  [128,2,65] psum pair; raw 65-col result DMA'd out; host divides.
PV(p-1) is emitted right after pair p's copies so the PE chews on it
while DVE copies land (lag-1 pipeline).
"""

import sys

import numpy as np
import ml_dtypes

try:  # concourse ships in the TRN container; fall back to its known path
    import concourse  # noqa: F401
except ImportError:
    sys.path.insert(0, "/opt/trn_rl_repo")

B, T, C, DK = 4, 2048, 1024, 64
NLI = 8          # query tiles per core
NSLOT = 16       # key tiles (slots) per batch

OWN = [0, 2, 4, 6, 8, 10, 12, 13]   # slot of own query tile li
CMP = [1, 3, 5, 7, 9, 11, 14, 15]   # slot of comp tile k

_CACHE = {}


def _build():
    import concourse.bacc as bacc
    import concourse.tile as tile
    import concourse.mybir as mybir

    f32 = mybir.dt.float32
    f16 = mybir.dt.float16
    bf16 = mybir.dt.bfloat16
    EXP = mybir.ActivationFunctionType.Exp
    CPY = mybir.ActivationFunctionType.Copy

    nc = bacc.Bacc("TRN2", target_bir_lowering=False, debug=False,
                   enable_asserts=False, num_devices=8)

    xt_d = nc.dram_tensor("xt", [128, NSLOT, 1024], f16,
                          kind="ExternalInput").ap()
    wa_d = nc.dram_tensor("wa", [128, 8, 2, 64], f16,
                          kind="ExternalInput").ap()
    wb_d = nc.dram_tensor("wb", [128, 8, 2, 64], f16,
                          kind="ExternalInput").ap()
    idt_d = nc.dram_tensor("idt", [64, 64], f16, kind="ExternalInput").ap()
    msk_d = nc.dram_tensor("msk", [128, 136], bf16,
                           kind="ExternalInput").ap()
    y_d = nc.dram_tensor("y", [128, NLI, DK + 1], f32,
                         kind="ExternalOutput").ap()

    with tile.TileContext(nc) as tc:
        with (
            tc.tile_pool(name="persist", bufs=1) as pp,
            tc.tile_pool(name="pmix", bufs=2, space="PSUM") as pmx,
            tc.tile_pool(name="ptrp", bufs=1, space="PSUM") as ptrp,
            tc.tile_pool(name="pst", bufs=2, space="PSUM") as pst,
            tc.tile_pool(name="pout", bufs=1, space="PSUM") as pou,
        ):
            xt = pp.tile([128, NSLOT, 1024], f16, tag="xt", name="xt")
            wa = pp.tile([128, 8, 2, 64], f16, tag="wa", name="wa")
            wb = pp.tile([128, 8, 2, 64], f16, tag="wb", name="wb")
            idt = pp.tile([64, 64], f16, tag="idt", name="idt")
            msk = pp.tile([128, 136], bf16, tag="msk", name="msk")
            tri = msk[:, 0:128]
            svec = pp.tile([128, NLI], f32, tag="svec", name="svec")
            kt = pp.tile([64, NSLOT, 128], f16, tag="kt", name="kt")
            # qvt[:, i, 0, :] = Q^T of own li=i; [:, i, 1, :] = V^T of
            # comp k=i (transpose staging)
            qvt = pp.tile([64, NLI, 2, 128], f16, tag="qvt", name="qvt")
            vv = pp.tile([128, NSLOT, DK + 1], bf16, tag="vv", name="vv")
            vz = pp.tile([128, NLI, DK + 1], bf16, tag="vz", name="vz")
            po_sb = pp.tile([128, NLI, DK + 1], f32, tag="posb", name="posb")
            scr = pp.tile([128, 128], f16, tag="scr", name="scr")
            E = [pp.tile([128, (2 * li + 2) * 128], bf16, tag=f"E{li}",
                         name=f"E{li}") for li in range(NLI)]

            nc.vector.memset(scr, 0.0)
            nc.vector.memset(vv[:, :, DK:DK + 1], 1.0)

            # ---- DMA program (all SP; transfers serialize in order) ----
            nc.sync.dma_start(wa, wa_d)
            nc.sync.dma_start(xt[:, 0, 0:512], xt_d[:, 0, 0:512])
            nc.sync.dma_start(xt[:, 0, 512:1024], xt_d[:, 0, 512:1024])
            nc.sync.dma_start(wb, wb_d)
            nc.sync.dma_start(xt[:, 1, :], xt_d[:, 1, :])
            nc.sync.dma_start(xt[:, 2:4, :], xt_d[:, 2:4, :])
            nc.sync.dma_start(msk, msk_d)
            nc.sync.dma_start(idt, idt_d)
            for a, b in [(4, 6), (6, 8), (8, 10), (10, 12),
                         (12, 14)]:
                nc.sync.dma_start(xt[:, a:b, :], xt_d[:, a:b, :])
            nc.sync.dma_start(xt[:, 14, :], xt_d[:, 14, :])
            nc.sync.dma_start(xt[:, 15, 0:512], xt_d[:, 15, 0:512])
            nc.sync.dma_start(xt[:, 15, 512:1024], xt_d[:, 15, 512:1024])

            # ---- minimal PE warmup (sets pe_busy_start ~1.2us) ----
            for _ in range(2):
                pw = pmx.tile([128, 3, 128], f32, tag="pmix", name="pw")
                nc.tensor.matmul(pw[:, 0, 0:2], scr, scr[:, 0:2],
                                 start=True, stop=True)

            ps_of = {}

            def ps_pair(p):
                if p not in ps_of:
                    ps_of[p] = pmx.tile([128, 3, 128], f32, tag="pmix",
                                        name=f"pp{p}")
                return ps_of[p]

            def fused(p, col, wt, s, chunks=tuple(range(8)), first=True,
                      last=True):
                ps = ps_pair(p)
                for i, ch in enumerate(chunks):
                    nc.tensor.matmul(
                        ps[:, col, :], wt[:, ch, :, :],
                        xt[:, s, ch * 128:(ch + 1) * 128],
                        start=(first and i == 0),
                        stop=(last and i == len(chunks) - 1))

            def vnat(p, s, chunks=tuple(range(8)), first=True, last=True):
                ps = ps_pair(p)
                for i, ch in enumerate(chunks):
                    nc.tensor.matmul(
                        ps[:, 2, 0:DK],
                        xt[:, s, ch * 128:(ch + 1) * 128],
                        wb[:, ch, 1, :],
                        start=(first and i == 0),
                        stop=(last and i == len(chunks) - 1))

            def cp_pair(p, vv_eng=None):
                """kt pair + q|v pair + vv-own copies for full pair p."""
                ps = ps_of[p]
                s0 = 2 * p
                nc.vector.tensor_copy(kt[:, s0:s0 + 2, :],
                                      ps[0:64, 0:2, :])
                nc.vector.tensor_copy(qvt[:, p, :, :], ps[64:128, 0:2, :])
                if vv_eng == "act":
                    nc.scalar.activation(vv[:, s0, 0:DK], ps[:, 2, 0:DK],
                                         CPY)
                else:
                    nc.vector.tensor_copy(vv[:, s0, 0:DK], ps[:, 2, 0:DK])

            def tr_comp(k):
                """PE transpose V^T(comp k) -> natural V in vv[CMP[k]]."""
                pt = ptrp.tile([128, 2, DK], f16, tag="ptrp", name=f"pt{k}")
                nc.tensor.transpose(pt[:, k % 2, :], qvt[:, k, 1, :], idt)
                nc.vector.tensor_copy(vv[:, CMP[k], 0:DK], pt[:, k % 2, :])

            def s_exp(li, blocks, tag=""):
                """S^T then exp for E[li] col blocks `blocks`."""
                nb = len(blocks)
                ps = pst.tile([128, 1024], f32, tag="pst",
                              name=f"ps{li}{tag}")
                for i, j in enumerate(blocks):
                    s = OWN[j] if j <= li else CMP[j - li - 1]
                    nc.tensor.matmul(
                        ps[:, i * 128:(i + 1) * 128],
                        kt[:, s, :],
                        qvt[:, li, 0, :],
                        start=True, stop=True,
                    )
                j0 = blocks[0]
                nc.scalar.activation(
                    E[li][:, j0 * 128:(j0 + nb) * 128], ps[:, 0:nb * 128],
                    EXP)

            def tri_mul(li):
                nc.gpsimd.tensor_mul(
                    E[li][:, li * 128:(li + 1) * 128],
                    E[li][:, li * 128:(li + 1) * 128], tri)

            def vz_make(li):
                nc.gpsimd.tensor_scalar_mul(
                    vz[:, li, :], vv[:, CMP[li], :], svec[:, li:li + 1])

            po_t = {}

            def pv_mm(li, blocks, start, stop):
                pr = li // 2
                if pr not in po_t:
                    po_t[pr] = pou.tile([128, 2, DK + 1], f32, tag="pout",
                                        name=f"po{pr}")
                po = po_t[pr][:, li % 2, :]
                last = blocks[-1]
                for j in blocks:
                    if j == 2 * li + 1:
                        rhs = vz[:, li, :]
                    else:
                        s = OWN[j] if j <= li else CMP[j - li - 1]
                        rhs = vv[:, s, :]
                    nc.tensor.matmul(
                        po, E[li][:, j * 128:(j + 1) * 128], rhs,
                        start=(start and j == blocks[0]),
                        stop=(stop and j == last),
                        skip_group_check=True,
                    )

            def pv(li):
                pv_mm(li, list(range(2 * li + 2)), True, True)

            def po_cp(li0, n):
                # copy n li-columns of PV psum to SBUF staging (DVE)
                pr = li0 // 2
                c0 = li0 % 2
                nc.vector.tensor_copy(po_sb[:, li0:li0 + n, :],
                                      po_t[pr][:, c0:c0 + n, :])

            # ================= main schedule =================
            # ACT table primer: forces LoadActFuncSet at t~0.7us instead
            # of behind the first data-dependent ACT instruction
            nc.scalar.activation(svec[:, 0:1], svec[:, 0:1], EXP)
            # pair 0 (slots 0, 1): own x arrives in halves
            fused(0, 0, wa, 0, (0, 1, 2, 3), True, False)
            fused(0, 0, wa, 0, (4, 5, 6, 7), False, True)
            vnat(0, 0)
            fused(0, 1, wb, 1)
            cp_pair(0, "act")
            nc.vector.tensor_copy(svec, msk[:, 128:136])  # bf16 -> f32
            # pair 1 (slots 2, 3); lagged work for pair 0
            fused(1, 0, wa, 2)
            fused(1, 1, wb, 3)
            vnat(1, 2)
            cp_pair(1, "act")
            s_exp(0, [0, 1])
            tr_comp(0)
            tri_mul(0)
            vz_make(0)
            # pair 2 (slots 4, 5); lagged pair 1; PV(0)
            fused(2, 0, wa, 4)
            fused(2, 1, wb, 5)
            vnat(2, 4)
            cp_pair(2, "act")
            s_exp(1, [0, 1, 2, 3])
            tr_comp(1)
            tri_mul(1)
            vz_make(1)
            pv(0)
            # pair 3 (slots 6, 7)
            fused(3, 0, wa, 6)
            fused(3, 1, wb, 7)
            vnat(3, 6)
            cp_pair(3, "act")
            s_exp(2, [0, 1, 2, 3, 4, 5])
            tr_comp(2)
            tri_mul(2)
            vz_make(2)
            pv(1)
            po_cp(0, 2)
            nc.sync.dma_start(y_d[:, 0:2, :], po_sb[:, 0:2, :])
            # pair 4 (slots 8, 9)
            fused(4, 0, wa, 8)
            fused(4, 1, wb, 9)
            vnat(4, 8)
            cp_pair(4, "act")
            s_exp(3, list(range(8)))
            tr_comp(3)
            tri_mul(3)
            vz_make(3)
            pv(2)
            # pair 5 (slots 10, 11)
            fused(5, 0, wa, 10)
            fused(5, 1, wb, 11)
            vnat(5, 10)
            cp_pair(5)
            s_exp(4, list(range(8)), "a")
            s_exp(4, [8, 9], "b")
            tr_comp(4)
            tri_mul(4)
            vz_make(4)
            pv(3)
            po_cp(2, 2)
            nc.sync.dma_start(y_d[:, 2:4, :], po_sb[:, 2:4, :])
            # own 6 (slot 12); lagged pair 5
            fused(6, 0, wa, 12)
            vnat(6, 12)
            ps6 = ps_of[6]
            nc.vector.tensor_copy(kt[:, 12, :], ps6[0:64, 0, :])
            nc.vector.tensor_copy(qvt[:, 6, 0, :], ps6[64:128, 0, :])
            nc.vector.tensor_copy(vv[:, 12, 0:DK], ps6[:, 2, 0:DK])
            s_exp(5, list(range(8)), "a")
            s_exp(5, [8, 9, 10, 11], "b")
            tr_comp(5)
            tri_mul(5)
            vz_make(5)
            pv(4)
            # own 7 (slot 13); E[6] minus block 13
            fused(7, 0, wa, 13)
            vnat(7, 13)
            ps7 = ps_of[7]
            nc.vector.tensor_copy(kt[:, 13, :], ps7[0:64, 0, :])
            nc.vector.tensor_copy(qvt[:, 7, 0, :], ps7[64:128, 0, :])
            nc.vector.tensor_copy(vv[:, 13, 0:DK], ps7[:, 2, 0:DK])
            s_exp(6, list(range(8)), "a")
            s_exp(6, [8, 9, 10, 11, 12], "b")
            tri_mul(6)
            pv(5)
            po_cp(4, 2)
            nc.sync.dma_start(y_d[:, 4:6, :], po_sb[:, 4:6, :])
            # comp 6 (slot 14); E[7] minus blocks 14, 15
            fused(8, 1, wb, 14)
            ps8 = ps_of[8]
            nc.vector.tensor_copy(kt[:, 14, :], ps8[0:64, 1, :])
            nc.vector.tensor_copy(qvt[:, 6, 1, :], ps8[64:128, 1, :])
            s_exp(7, list(range(8)), "a")
            s_exp(7, [8, 9, 10, 11, 12, 13], "b")
            tri_mul(7)
            s_exp(6, [13], "c")
            tr_comp(6)
            vz_make(6)
            pv(6)
            po_cp(6, 1)
            nc.sync.dma_start(y_d[:, 6:7, :], po_sb[:, 6:7, :])
            s_exp(7, [14], "c")
            pv_mm(7, list(range(15)), True, False)
            # comp 7 (slot 15, split by x halves)
            fused(9, 1, wb, 15, (0, 1, 2, 3), True, False)
            fused(9, 1, wb, 15, (4, 5, 6, 7), False, True)
            ps9 = ps_of[9]
            nc.vector.tensor_copy(kt[:, 15, :], ps9[0:64, 1, :])
            nc.vector.tensor_copy(qvt[:, 7, 1, :], ps9[64:128, 1, :])
            s_exp(7, [15], "d")
            tr_comp(7)
            vz_make(7)
            pv_mm(7, [15], False, True)
            po_cp(7, 1)
            nc.sync.dma_start(y_d[:, 7:8, :], po_sb[:, 7:8, :])

    nc.compile()
    return nc


def _host_inputs(x, Wq, Wk, Wv):
    """Per-core input maps. Core c = 2*b + jj."""
    x16 = x.astype(np.float16)
    wk16 = Wk.astype(np.float16).reshape(8, 128, DK).transpose(1, 0, 2)
    wq16 = Wq.astype(np.float16).reshape(8, 128, DK).transpose(1, 0, 2)
    wv16 = (Wv / 8.0).astype(np.float16).reshape(8, 128, DK).transpose(
        1, 0, 2)
    wa_h = np.empty((128, 8, 2, DK), dtype=np.float16)
    wa_h[:, :, 0, :] = wk16
    wa_h[:, :, 1, :] = wq16
    wb_h = np.empty((128, 8, 2, DK), dtype=np.float16)
    wb_h[:, :, 0, :] = wk16
    wb_h[:, :, 1, :] = wv16
    idt = np.eye(64, dtype=np.float16)
    tri = (np.arange(128)[:, None] <= np.arange(128)[None, :])
    in_maps = []
    for core in range(8):
        b, jj = divmod(core, 2)
        sel = [int(k >= 4) if jj == 0 else int(k < 4) for k in range(8)]
        g = [2 * k + sel[k] for k in range(8)]
        cg = [2 * k + 1 - sel[k] for k in range(8)]
        slot_order = [0] * NSLOT
        for li in range(NLI):
            slot_order[OWN[li]] = g[li]
        for k in range(NLI):
            slot_order[CMP[k]] = cg[k]
        arr = x16[b].reshape(16, 128, 8, 128)         # [tile, r, ch, p]
        xt = np.ascontiguousarray(
            arr[slot_order].transpose(3, 0, 2, 1).reshape(128, NSLOT, 1024))
        msk = np.zeros((128, 136), dtype=np.float32)
        msk[:, 0:128] = tri
        msk[:, 128:136] = np.asarray(sel, dtype=np.float32)
        in_maps.append({
            "xt": xt,
            "wa": wa_h,
            "wb": wb_h,
            "idt": idt,
            "msk": msk.astype(ml_dtypes.bfloat16),
        })
    return in_maps


def kernel(x, Wq, Wk, Wv):
    from concourse.bass_utils import run_bass_kernel_spmd

    x = np.asarray(x, dtype=np.float32)
    Wq = np.asarray(Wq, dtype=np.float32)
    Wk = np.asarray(Wk, dtype=np.float32)
    Wv = np.asarray(Wv, dtype=np.float32)

    if "nc" not in _CACHE:
        _CACHE["nc"] = _build()
    nc = _CACHE["nc"]

    in_maps = _host_inputs(x, Wq, Wk, Wv)
    res = run_bass_kernel_spmd(nc, in_maps, core_ids=list(range(8)))
    out = np.empty((B, T, DK), dtype=np.float32)
    for core in range(8):
        b, jj = divmod(core, 2)
        sel = [int(k >= 4) if jj == 0 else int(k < 4) for k in range(8)]
        yloc = res.results[core]["y"]                 # [128, 8, 65]
        for li in range(NLI):
            gt = 2 * li + sel[li]
            out[b, gt * 128:(gt + 1) * 128, :] = (
                yloc[:, li, 0:DK] / yloc[:, li, DK:DK + 1])
    return out
